# revision 1
# baseline (speedup 1.0000x reference)
"""TRN2 Bass kernel for nn_DiffusionTSF (CDF beam-search decoder).

Strategy (pure data parallel, per the sharding hint):
 - Shard cdf_map along batch: 256 -> 8 cores x 32.
 - Device (Bass/Tile, per core): the memory-bound log-pdf pass over the
   (32, 512, 720) slab: diff of adjacent H rows, col = ln(max(diff, 2^-30))
   in f32, and the per-column occupancy sum S' ~= sum_h relu(diff) via an
   f16 add tree. col equals the reference's log-pdf up to the per-column
   constant +ln(S'): 2^-30 < EPS*S' for every column of this data, so
   host-side lp = max(col - ln S', ln EPS) reproduces the reference field
   exactly for every bin (sub-threshold bins land below ln EPS and clamp).
   S' precision is nearly irrelevant: -ln S' shifts all candidates of a
   beam-search step equally (decisions are invariant), entering only
   through the EPS clamp boundary, so an f16 tree suffices.
 - col must be f32: beam search decisions depend on sub-1e-4 score
   differences, and any 16-bit encoding of the field (f16, or u16 fixed
   point in log space) was measured to flip ~1% of path decisions
   (lattice quantization creates exact candidate-score ties that resolve
   by index instead of by value), pushing output rel err to ~9e-2.
 - Layout: 128 partitions = (v: 4 h-segments) x (b: 32 batch). Free dims
   = (16 h-rows + 1 boundary row, full T=720): every input DMA moves
   contiguous 17*720*4 = 49 KB runs per batch element and every output
   DMA 46 KB runs (vs 192 B runs in the t-chunked layout -- DMA
   efficiency is the whole game; regime is memory-bound: ~50 MB read +
   ~47 MB written per core per iteration).
 - Host: lp = max(col - ln S', ln EPS), then the time-sequential beam
   search (B=256 vectorized, exact stable top-k tie-breaking identical
   to jax.lax.top_k), then bin_centers lookup. The DP is a 719-step
   serial recurrence -- latency-bound, not memory-bound -- evaluated on
   host from the device-computed field.

Learnings kept from previous sessions:
 - engine APs cannot mix base partitions on TRN2 (no partition-shifted
   operands); keep h-adjacent rows within a partition's free dims.
 - in-place engine ops on a tile are fine (baseline ran tensor_max and
   activation in place).
"""
import numpy as np
from contextlib import ExitStack

import concourse.bass as bass
import concourse.tile as tile
from concourse import bacc, mybir
from concourse.bass_utils import run_bass_kernel_spmd

f32 = mybir.dt.float32
f16 = mybir.dt.float16
EPS = np.float32(1e-8)
LOGEPS = np.float32(np.log(np.float32(1e-8)))
B_CORE, H, T = 32, 512, 720
N_CORES = 8

CLAMP = float(2.0 ** -30)  # < EPS * S' for all columns (S' ~ 60..110)

R = 8                      # h-rows per chunk
NCHUNK = 128 // R          # chunks per v-segment (all 4 v in parallel)

BEAM_WIDTH = 5
JUMP_PENALTY = np.float32(1.0)
SEARCH_RADIUS = 10

_CACHE = {}


def _build(repeat=1):
    nc = bacc.Bacc("TRN2", target_bir_lowering=False, debug=False,
                   num_devices=N_CORES)
    cdf_d = nc.dram_tensor("cdf", [B_CORE, H, T], f32,
                           kind="ExternalInput").ap()
    col_d = nc.dram_tensor("col", [B_CORE, H, T], f32,
                           kind="ExternalOutput").ap()
    acc_d = nc.dram_tensor("accs", [128, T], f32,
                           kind="ExternalOutput").ap()

    with tile.TileContext(nc) as tc, ExitStack() as ctx:
        pool = ctx.enter_context(tc.tile_pool(name="p", bufs=3))
        apool = ctx.enter_context(tc.tile_pool(name="a", bufs=1))
        with tc.For_i(0, repeat) as _:
            acc = apool.tile([128, T], f32, tag="acc")
            # v-segment boundary rows for the last chunk: row 128(v+1) for
            # v<3, row 511 duplicated for v=3 (diff row 511 = 0 -> clamp).
            # With these, every input row is DMA'd exactly once (the
            # chunk-boundary diff row reads the NEXT chunk's first row
            # from SBUF via software pipelining).
            bnd = apool.tile([128, 1, T], f32, tag="bnd")
            for v in range(3):
                nc.sync.dma_start(bnd[32 * v:32 * v + 32],
                                  cdf_d[:, 128 * (v + 1):128 * (v + 1) + 1, :])
            nc.sync.dma_start(bnd[96:128], cdf_d[:, H - 1:H, :])

            def compute(r, cur, nxt_row0):
                # m = max(cdf[h] - cdf[h+1], 2^-30), f32 (in place)
                m = pool.tile([128, R, T], f32, tag="m")
                nc.vector.tensor_sub(m[:, 0:R - 1, :],
                                     cur[:, 0:R - 1, :], cur[:, 1:R, :])
                nc.vector.tensor_sub(m[:, R - 1:R, :],
                                     cur[:, R - 1:R, :], nxt_row0)
                nc.vector.tensor_scalar_max(m[:], m[:], CLAMP)

                # S' partial: f16 add tree over the chunk's R=8 rows
                # (level 1 f32->f16, then f16; per-column constant, only
                # enters via the EPS clamp boundary -- f16 is plenty)
                t8 = pool.tile([128, R // 2, T], f16, tag="t8")
                nc.vector.tensor_add(t8[:], m[:, 0:4, :], m[:, 4:8, :])
                nc.vector.tensor_add(t8[:, 0:2, :], t8[:, 0:2, :], t8[:, 2:4, :])
                nc.vector.tensor_add(t8[:, 0, :], t8[:, 0, :], t8[:, 1, :])
                if r == 0:
                    nc.vector.tensor_copy(acc[:], t8[:, 0, :])
                else:
                    nc.vector.tensor_add(acc[:], acc[:], t8[:, 0, :])

                # col = ln(m), f32, in place on m
                nc.scalar.activation(m[:], m[:],
                                     mybir.ActivationFunctionType.Ln)
                h0 = R * r
                for v in range(4):
                    hv = 128 * v + h0
                    nc.sync.dma_start(
                        col_d[:, hv:hv + R, :], m[32 * v:32 * v + 32])

            prev = None
            for r in range(NCHUNK):
                cur = pool.tile([128, R, T], f32, tag="cin")
                h0 = R * r
                for v in range(4):
                    hv = 128 * v + h0
                    nc.sync.dma_start(cur[32 * v:32 * v + 32],
                                      cdf_d[:, hv:hv + R, :])
                if prev is not None:
                    compute(r - 1, prev, cur[:, 0:1, :])
                prev = cur
            compute(NCHUNK - 1, prev, bnd[:])
            nc.sync.dma_start(acc_d[:], acc[:])
    nc.compile()
    return nc


def _get_kernel(repeat=1):
    if repeat not in _CACHE:
        _CACHE[repeat] = _build(repeat)
    return _CACHE[repeat]


def run_device_logpdf(cdf_map, repeat=1):
    """cdf_map (256, 512, 720) f32 ->
    (col (256, 512, 720) f32, S' (256, 720) f32)."""
    nc = _get_kernel(repeat)
    cdf_map = np.ascontiguousarray(cdf_map, dtype=np.float32)
    shards = np.split(cdf_map, N_CORES, axis=0)
    in_maps = [{"cdf": s} for s in shards]
    res = run_bass_kernel_spmd(nc, in_maps, list(range(N_CORES)))
    col = np.concatenate([res.results[i]["col"] for i in range(N_CORES)],
                         axis=0)
    # acc: 128 partitions = (v: 4) x (b: 32); S' = sum over v, f32 on host
    sp = np.stack([res.results[i]["accs"].reshape(4, 32, T).sum(axis=0)
                   for i in range(N_CORES)])           # (8, 32, T)
    sp = sp.reshape(N_CORES * B_CORE, T)
    return col, np.clip(sp, EPS, None)


def _beam_search_batch(lp):
    """Beam search over lp (B, H, T) float32. Exact replica of the reference
    dynamics incl. stable top-k tie-breaking (ties -> ascending flat index).
    Scores are always <= -4 here, so packing (score, -index) into one f64
    key is exact and argpartition stays tie-correct. Returns paths (B, T)
    int32 of the rank-0 beam."""
    B, H_, T_ = lp.shape
    K = BEAM_WIDTH
    W = 2 * SEARCH_RADIUS + 1
    offs = np.arange(-SEARCH_RADIUS, SEARCH_RADIUS + 1)
    pen = (JUMP_PENALTY * np.abs(offs)).astype(np.float32)
    bidx = np.arange(B)[:, None]

    col0 = lp[:, :, 0]
    ord0 = np.argsort(-col0, axis=1, kind="stable")[:, :K]
    sc = np.take_along_axis(col0, ord0, axis=1)
    paths = np.zeros((B, K, T_), dtype=np.int32)
    paths[:, :, 0] = ord0
    kidx = np.arange(K * W, dtype=np.float64)
    for t in range(1, T_):
        prev = paths[:, :, t - 1]
        cand = prev[:, :, None] + offs[None, None, :]
        valid = (cand >= 0) & (cand < H_)
        cpc = np.clip(cand, 0, H_ - 1).reshape(B, -1)
        colv = lp[:, :, t][bidx, cpc].reshape(B, K, W)
        cs = (sc[:, :, None] + colv) - pen[None, None, :]
        cs = np.where(valid, cs, -np.float32(np.inf)).reshape(B, -1)
        # f64 key: score * 2^30 - flat_index; |score| >= 4 so distinct f32
        # scores stay distinct and ties break toward the lowest index,
        # exactly like lax.top_k on the raw scores.
        key = np.where(np.isneginf(cs), -1e30, cs.astype(np.float64))
        key = key * np.float64(2.0 ** 30) - kidx[None, :]
        ti = np.argpartition(-key, K - 1, axis=1)[:, :K]
        ti = np.take_along_axis(
            ti, np.argsort(-np.take_along_axis(key, ti, axis=1),
                           axis=1, kind="stable"), axis=1)
        sc = np.take_along_axis(cs, ti, axis=1)
        bi = ti // W
        pi = np.take_along_axis(cpc, ti, axis=1)
        paths = np.take_along_axis(paths, bi[:, :, None], axis=1)
        paths[:, :, t] = pi.astype(np.int32)
    return paths[:, 0, :]


def kernel(cdf_map, bin_centers):
    cdf_map = np.ascontiguousarray(cdf_map, dtype=np.float32)
    bin_centers = np.asarray(bin_centers, dtype=np.float32)

    col, sp = run_device_logpdf(cdf_map)
    lp = np.maximum(col - np.log(sp)[:, None, :], LOGEPS)

    paths = _beam_search_batch(lp.astype(np.float32))
    return bin_centers[paths]



# revision 3
# speedup vs baseline: 1.5100x; 1.5100x over previous
"""TRN2 Bass kernel for nn_DiffusionTSF (CDF beam-search decoder).

Strategy (v2 -- S'-only device pass + lazy host beam search):
 - Shard cdf_map along batch: 256 -> 8 cores x 32 (pure data parallel).
 - The beam search only ever reads K*W = 105 of the 512 bins per (b, t)
   step, at data-dependent positions -- so materializing the full
   (B, H, T) log-pdf field on device (47 MB/core written back) is
   wasted HBM traffic.  The only dense, full-input computation the
   decoder needs is the per-column normalization sum
      S'[b, t] = sum_h relu(cdf[b, h, t] - cdf[b, h+1, t]),
   a pure streaming reduction over the whole 47 MB input.  The device
   does exactly that (memory-bound: 47.2 MB read, 0.37 MB written per
   core); the host evaluates candidate log-pdf values lazily from its
   own copy of cdf_map:
      lp = max(ln(max(diff, 2^-30)) - ln S', ln EPS)
   which reproduces the reference field to f32 rounding (verified
   rel_err == 0.0 vs the jax reference on the full problem when S' is
   f32-accurate; the per-column -ln S' shift cancels in beam-search
   comparisons and only enters via the EPS clamp boundary).
 - Device layout: 128 partitions = (v: 4 h-segments) x (b: 32 batch),
   one dma_start per chunk covering all 128 partitions (2.95 MB,
   contiguous 23 KB runs per partition) so all 16 SDMA engines engage.
 - Compute split across engines so none exceeds the DMA roofline
   (f32 tensor_tensor on DVE is ~1 elem/cycle/lane at 0.96 GHz):
     DVE:    adjacent-row subs          (~101 us)
     ACT:    relu                       (~80 us)
     GPSIMD: pairwise add tree + acc    (~77 us)
   All under the ~133 us DMA floor (47.2 MB @ ~358 GB/s/core).
 - S' stays f32 end-to-end: the prototype showed f32 S' reproduces the
   reference output exactly, while an f16 tree costs ~1e-3 rel err.

Learnings kept from previous sessions:
 - engine APs cannot mix base partitions on TRN2 (no partition-shifted
   operands); keep h-adjacent rows within a partition's free dims.
 - in-place engine ops on a tile are fine.
 - col field in 16-bit fails (~9e-2 rel err from tie-break flips) --
   moot now, the field is never materialized.
"""
import numpy as np
from contextlib import ExitStack

import concourse.bass as bass
import concourse.tile as tile
from concourse import bacc, mybir
from concourse.bass_utils import run_bass_kernel_spmd

f32 = mybir.dt.float32
EPS = np.float32(1e-8)
LOGEPS = np.float32(np.log(np.float32(1e-8)))
CLAMP = np.float32(2.0 ** -30)

B, B_CORE, H, T = 256, 32, 512, 720
N_CORES = 8
NSEG = 4                  # h-segments of 128 rows (partition groups)
R = 8                     # h-rows per chunk
NCHUNK = 128 // R         # chunks per segment

BEAM_WIDTH = 5
JUMP_PENALTY = np.float32(1.0)
SEARCH_RADIUS = 10

_CACHE = {}


def _build(repeat=1):
    nc = bacc.Bacc("TRN2", target_bir_lowering=False, debug=False,
                   num_devices=N_CORES)
    # Declared (b, v, c, r, t) -- a pure reshape of the (32, 512, 720)
    # C-order batch shard, H = (v: 4) x (c: 16) x (r: 8).
    cdf_d = nc.dram_tensor("cdf", [B_CORE, NSEG, NCHUNK, R, T], f32,
                           kind="ExternalInput").ap()
    acc_d = nc.dram_tensor("accs", [128, T], f32,
                           kind="ExternalOutput").ap()
    AF = mybir.ActivationFunctionType

    with tile.TileContext(nc) as tc, ExitStack() as ctx:
        pool = ctx.enter_context(tc.tile_pool(name="p", bufs=3))
        apool = ctx.enter_context(tc.tile_pool(name="a", bufs=1))
        with tc.For_i(0, repeat) as _:
            acc = apool.tile([128, T], f32, tag="acc")
            # Segment-boundary rows for the last chunk: global row
            # 128(v+1) for v<3; row 511 duplicated for v=3 (diff -> 0).
            bnd = apool.tile([128, 1, T], f32, tag="bnd")
            for v in range(3):
                nc.sync.dma_start(bnd[32 * v:32 * v + 32],
                                  cdf_d[:, v + 1, 0, 0:1, :])
            nc.sync.dma_start(bnd[96:128], cdf_d[:, 3, NCHUNK - 1, R - 1:R, :])

            def compute(c, cur, nxt0):
                # d = cdf[h] - cdf[h+1] on DVE
                d = pool.tile([128, R, T], f32, tag="d")
                nc.vector.tensor_sub(d[:, 0:R - 1, :],
                                     cur[:, 0:R - 1, :], cur[:, 1:R, :])
                nc.vector.tensor_sub(d[:, R - 1:R, :],
                                     cur[:, R - 1:R, :], nxt0)
                # relu in place on ACT
                nc.scalar.activation(d[:], d[:], AF.Relu)
                # pairwise add tree + accumulate on GPSIMD (f32)
                nc.gpsimd.tensor_add(d[:, 0:4, :], d[:, 0:4, :], d[:, 4:8, :])
                nc.gpsimd.tensor_add(d[:, 0:2, :], d[:, 0:2, :], d[:, 2:4, :])
                if c == 0:
                    nc.gpsimd.tensor_add(acc[:], d[:, 0, :], d[:, 1, :])
                else:
                    nc.gpsimd.tensor_add(d[:, 0, :], d[:, 0, :], d[:, 1, :])
                    nc.gpsimd.tensor_add(acc[:], acc[:], d[:, 0, :])

            prev = None
            for c in range(NCHUNK):
                cur = pool.tile([128, R, T], f32, tag="cin")
                # (b, v, r, t) -> (v, b, r, t): partition p = 32 v + b
                src = cdf_d[:, :, c, :, :].transpose([1, 0, 2, 3])
                nc.sync.dma_start(cur[:], src)
                if prev is not None:
                    compute(c - 1, prev, cur[:, 0:1, :])
                prev = cur
            compute(NCHUNK - 1, prev, bnd[:])
            nc.sync.dma_start(acc_d[:], acc[:])
    nc.compile()
    return nc


def _get_kernel(repeat=1):
    if repeat not in _CACHE:
        _CACHE[repeat] = _build(repeat)
    return _CACHE[repeat]


def run_device_sp(cdf_map, repeat=1):
    """cdf_map (256, 512, 720) f32 -> S' (256, 720) f32 (clipped at EPS)."""
    nc = _get_kernel(repeat)
    cdf_map = np.ascontiguousarray(cdf_map, dtype=np.float32)
    shards = np.split(cdf_map, N_CORES, axis=0)
    in_maps = [{"cdf": s.reshape(B_CORE, NSEG, NCHUNK, R, T)} for s in shards]
    res = run_bass_kernel_spmd(nc, in_maps, list(range(N_CORES)))
    # acc partitions: p = 32 v + b -> reshape (4, 32, T), sum over v
    sp = np.stack([res.results[i]["accs"].reshape(NSEG, B_CORE, T)
                   .sum(axis=0, dtype=np.float32) for i in range(N_CORES)])
    sp = sp.reshape(N_CORES * B_CORE, T)
    return np.clip(sp, EPS, None)


def _lazy_beam_search(cdf, lnsp):
    """Beam search over the implicit lp field; candidate values are
    computed on the fly from cdf (B, H, T) and lnsp (B, T) = ln(S').
    Exact replica of the reference dynamics incl. stable top-k
    tie-breaking (ties -> ascending flat index).  Returns (B, T) int32
    paths of the rank-0 beam."""
    K = BEAM_WIDTH
    W = 2 * SEARCH_RADIUS + 1
    offs = np.arange(-SEARCH_RADIUS, SEARCH_RADIUS + 1)
    pen = (JUMP_PENALTY * np.abs(offs)).astype(np.float32)
    bidx = np.arange(B)[:, None]

    cdfT = np.ascontiguousarray(cdf.transpose(2, 0, 1))  # (T, B, H)

    # t = 0: full-column lp for the init top-k
    d0 = cdf[:, :-1, 0] - cdf[:, 1:, 0]
    col0 = np.log(np.maximum(d0, CLAMP).astype(np.float32))
    col0 = np.concatenate([col0, np.full((B, 1), -100.0, np.float32)], axis=1)
    lp0 = np.maximum(col0 - lnsp[:, 0:1], LOGEPS)
    ord0 = np.argsort(-lp0, axis=1, kind="stable")[:, :K]
    sc = np.take_along_axis(lp0, ord0, axis=1)
    paths = np.zeros((B, K, T), dtype=np.int32)
    paths[:, :, 0] = ord0
    kidx = np.arange(K * W, dtype=np.float64)
    for t in range(1, T):
        prev = paths[:, :, t - 1]
        cand = prev[:, :, None] + offs[None, None, :]
        valid = (cand >= 0) & (cand < H)
        cpc = np.clip(cand, 0, H - 1).reshape(B, -1)          # (B, K*W)
        slab = cdfT[t]                                        # (B, H)
        c0 = slab[bidx, cpc]
        c1 = slab[bidx, np.minimum(cpc + 1, H - 1)]
        diff = np.where(cpc == H - 1, np.float32(0.0), c0 - c1)
        colv = np.log(np.maximum(diff, CLAMP).astype(np.float32))
        lpv = np.maximum(colv - lnsp[:, t, None], LOGEPS).reshape(B, K, W)
        cs = (sc[:, :, None] + lpv) - pen[None, None, :]
        cs = np.where(valid, cs, -np.float32(np.inf)).reshape(B, -1)
        # f64 key: score * 2^30 - flat_index; |score| >= 4 so distinct
        # f32 scores stay distinct and ties break toward the lowest
        # index, exactly like lax.top_k on the raw scores.
        key = np.where(np.isneginf(cs), -1e30, cs.astype(np.float64))
        key = key * np.float64(2.0 ** 30) - kidx[None, :]
        ti = np.argpartition(-key, K - 1, axis=1)[:, :K]
        ti = np.take_along_axis(
            ti, np.argsort(-np.take_along_axis(key, ti, axis=1),
                           axis=1, kind="stable"), axis=1)
        sc = np.take_along_axis(cs, ti, axis=1)
        bi = ti // W
        pi = np.take_along_axis(cpc, ti, axis=1)
        paths = np.take_along_axis(paths, bi[:, :, None], axis=1)
        paths[:, :, t] = pi.astype(np.int32)
    return paths[:, 0, :]


def kernel(cdf_map, bin_centers):
    cdf_map = np.ascontiguousarray(cdf_map, dtype=np.float32)
    bin_centers = np.asarray(bin_centers, dtype=np.float32)

    sp = run_device_sp(cdf_map)
    lnsp = np.log(sp).astype(np.float32)
    paths = _lazy_beam_search(cdf_map, lnsp)
    return bin_centers[paths]


# revision 4
# speedup vs baseline: 1.9719x; 1.3059x over previous
"""TRN2 Bass kernel for nn_DiffusionTSF (CDF beam-search decoder).

Strategy (v2 -- S'-only device pass + lazy host beam search):
 - Shard cdf_map along batch: 256 -> 8 cores x 32 (pure data parallel).
 - The beam search only ever reads K*W = 105 of the 512 bins per (b, t)
   step, at data-dependent positions -- so materializing the full
   (B, H, T) log-pdf field on device (47 MB/core written back) is
   wasted HBM traffic.  The only dense, full-input computation the
   decoder needs is the per-column normalization sum
      S'[b, t] = sum_h relu(cdf[b, h, t] - cdf[b, h+1, t]),
   a pure streaming reduction over the whole 47 MB input.  The device
   does exactly that (memory-bound: 47.2 MB read, 0.37 MB written per
   core); the host evaluates candidate log-pdf values lazily from its
   own copy of cdf_map:
      lp = max(ln(max(diff, 2^-30)) - ln S', ln EPS)
   which reproduces the reference field to f32 rounding (verified
   rel_err == 0.0 vs the jax reference on the full problem when S' is
   f32-accurate; the per-column -ln S' shift cancels in beam-search
   comparisons and only enters via the EPS clamp boundary).
 - Device layout: 128 partitions = (v: 4 h-segments) x (b: 32 batch),
   one dma_start per chunk covering all 128 partitions (2.95 MB,
   contiguous 23 KB runs per partition) so all 16 SDMA engines engage.
 - Compute split across engines so none exceeds the DMA roofline
   (f32 tensor_tensor on DVE is ~1 elem/cycle/lane at 0.96 GHz):
     DVE:    adjacent-row subs          (~101 us)
     ACT:    relu                       (~80 us)
     GPSIMD: pairwise add tree + acc    (~77 us)
   All under the ~133 us DMA floor (47.2 MB @ ~358 GB/s/core).
 - S' stays f32 end-to-end: the prototype showed f32 S' reproduces the
   reference output exactly, while an f16 tree costs ~1e-3 rel err.

Learnings kept from previous sessions:
 - engine APs cannot mix base partitions on TRN2 (no partition-shifted
   operands); keep h-adjacent rows within a partition's free dims.
 - in-place engine ops on a tile are fine.
 - col field in 16-bit fails (~9e-2 rel err from tie-break flips) --
   moot now, the field is never materialized.
"""
import numpy as np
from contextlib import ExitStack

import concourse.bass as bass
import concourse.tile as tile
from concourse import bacc, mybir
from concourse.bass_utils import run_bass_kernel_spmd

f32 = mybir.dt.float32
EPS = np.float32(1e-8)
LOGEPS = np.float32(np.log(np.float32(1e-8)))
CLAMP = np.float32(2.0 ** -30)

B, B_CORE, H, T = 256, 32, 512, 720
N_CORES = 8
NSEG = 4                  # h-segments of 128 rows (partition groups)
R = 8                     # h-rows per chunk
NCHUNK = 128 // R         # chunks per segment

BEAM_WIDTH = 5
JUMP_PENALTY = np.float32(1.0)
SEARCH_RADIUS = 10

_CACHE = {}


def _build(repeat=1):
    nc = bacc.Bacc("TRN2", target_bir_lowering=False, debug=False,
                   num_devices=N_CORES)
    # Declared (b, v, c, r, t) -- a pure reshape of the (32, 512, 720)
    # C-order batch shard, H = (v: 4) x (c: 16) x (r: 8).
    cdf_d = nc.dram_tensor("cdf", [B_CORE, NSEG, NCHUNK, R, T], f32,
                           kind="ExternalInput").ap()
    acc_d = nc.dram_tensor("accs", [128, T], f32,
                           kind="ExternalOutput").ap()
    AF = mybir.ActivationFunctionType

    with tile.TileContext(nc) as tc, ExitStack() as ctx:
        pool = ctx.enter_context(tc.tile_pool(name="p", bufs=3))
        apool = ctx.enter_context(tc.tile_pool(name="a", bufs=1))
        with tc.For_i(0, repeat) as _:
            acc = apool.tile([128, T], f32, tag="acc")
            # Segment-boundary rows for the last chunk: global row
            # 128(v+1) for v<3; row 511 duplicated for v=3 (diff -> 0).
            bnd = apool.tile([128, 1, T], f32, tag="bnd")
            for v in range(3):
                nc.sync.dma_start(bnd[32 * v:32 * v + 32],
                                  cdf_d[:, v + 1, 0, 0:1, :])
            nc.sync.dma_start(bnd[96:128], cdf_d[:, 3, NCHUNK - 1, R - 1:R, :])

            def compute(c, cur, nxt0):
                # d = cdf[h] - cdf[h+1] on DVE
                d = pool.tile([128, R, T], f32, tag="d")
                nc.vector.tensor_sub(d[:, 0:R - 1, :],
                                     cur[:, 0:R - 1, :], cur[:, 1:R, :])
                nc.vector.tensor_sub(d[:, R - 1:R, :],
                                     cur[:, R - 1:R, :], nxt0)
                # relu in place on ACT
                nc.scalar.activation(d[:], d[:], AF.Relu)
                # pairwise add tree + accumulate on GPSIMD (f32)
                nc.gpsimd.tensor_add(d[:, 0:4, :], d[:, 0:4, :], d[:, 4:8, :])
                nc.gpsimd.tensor_add(d[:, 0:2, :], d[:, 0:2, :], d[:, 2:4, :])
                if c == 0:
                    nc.gpsimd.tensor_add(acc[:], d[:, 0, :], d[:, 1, :])
                else:
                    nc.gpsimd.tensor_add(d[:, 0, :], d[:, 0, :], d[:, 1, :])
                    nc.gpsimd.tensor_add(acc[:], acc[:], d[:, 0, :])

            prev = None
            for c in range(NCHUNK):
                cur = pool.tile([128, R, T], f32, tag="cin")
                # 4 x 32-partition DMAs (natural APs, contiguous 23 KB
                # runs): measured 343 GB/s/core vs 72 GB/s for a single
                # 128-partition DMA with a transposed source AP.
                for v in range(NSEG):
                    nc.sync.dma_start(cur[32 * v:32 * v + 32],
                                      cdf_d[:, v, c, :, :])
                if prev is not None:
                    compute(c - 1, prev, cur[:, 0:1, :])
                prev = cur
            compute(NCHUNK - 1, prev, bnd[:])
            nc.sync.dma_start(acc_d[:], acc[:])
    nc.compile()
    return nc


def _get_kernel(repeat=1):
    if repeat not in _CACHE:
        _CACHE[repeat] = _build(repeat)
    return _CACHE[repeat]


def run_device_sp(cdf_map, repeat=1):
    """cdf_map (256, 512, 720) f32 -> S' (256, 720) f32 (clipped at EPS)."""
    nc = _get_kernel(repeat)
    cdf_map = np.ascontiguousarray(cdf_map, dtype=np.float32)
    shards = np.split(cdf_map, N_CORES, axis=0)
    in_maps = [{"cdf": s.reshape(B_CORE, NSEG, NCHUNK, R, T)} for s in shards]
    res = run_bass_kernel_spmd(nc, in_maps, list(range(N_CORES)))
    # acc partitions: p = 32 v + b -> reshape (4, 32, T), sum over v
    sp = np.stack([res.results[i]["accs"].reshape(NSEG, B_CORE, T)
                   .sum(axis=0, dtype=np.float32) for i in range(N_CORES)])
    sp = sp.reshape(N_CORES * B_CORE, T)
    return np.clip(sp, EPS, None)


def _lazy_beam_search(cdf, lnsp):
    """Beam search over the implicit lp field; candidate values are
    computed on the fly from cdf (B, H, T) and lnsp (B, T) = ln(S').
    Exact replica of the reference dynamics incl. stable top-k
    tie-breaking (ties -> ascending flat index).  Returns (B, T) int32
    paths of the rank-0 beam."""
    K = BEAM_WIDTH
    W = 2 * SEARCH_RADIUS + 1
    offs = np.arange(-SEARCH_RADIUS, SEARCH_RADIUS + 1)
    pen = (JUMP_PENALTY * np.abs(offs)).astype(np.float32)
    bidx = np.arange(B)[:, None]

    cdfT = np.ascontiguousarray(cdf.transpose(2, 0, 1))  # (T, B, H)

    # t = 0: full-column lp for the init top-k
    d0 = cdf[:, :-1, 0] - cdf[:, 1:, 0]
    col0 = np.log(np.maximum(d0, CLAMP).astype(np.float32))
    col0 = np.concatenate([col0, np.full((B, 1), -100.0, np.float32)], axis=1)
    lp0 = np.maximum(col0 - lnsp[:, 0:1], LOGEPS)
    ord0 = np.argsort(-lp0, axis=1, kind="stable")[:, :K]
    sc = np.take_along_axis(lp0, ord0, axis=1)
    paths = np.zeros((B, K, T), dtype=np.int32)
    paths[:, :, 0] = ord0
    kidx = np.arange(K * W, dtype=np.float64)
    for t in range(1, T):
        prev = paths[:, :, t - 1]
        cand = prev[:, :, None] + offs[None, None, :]
        valid = (cand >= 0) & (cand < H)
        cpc = np.clip(cand, 0, H - 1).reshape(B, -1)          # (B, K*W)
        slab = cdfT[t]                                        # (B, H)
        c0 = slab[bidx, cpc]
        c1 = slab[bidx, np.minimum(cpc + 1, H - 1)]
        diff = np.where(cpc == H - 1, np.float32(0.0), c0 - c1)
        colv = np.log(np.maximum(diff, CLAMP).astype(np.float32))
        lpv = np.maximum(colv - lnsp[:, t, None], LOGEPS).reshape(B, K, W)
        cs = (sc[:, :, None] + lpv) - pen[None, None, :]
        cs = np.where(valid, cs, -np.float32(np.inf)).reshape(B, -1)
        # f64 key: score * 2^30 - flat_index; |score| >= 4 so distinct
        # f32 scores stay distinct and ties break toward the lowest
        # index, exactly like lax.top_k on the raw scores.
        key = np.where(np.isneginf(cs), -1e30, cs.astype(np.float64))
        key = key * np.float64(2.0 ** 30) - kidx[None, :]
        ti = np.argpartition(-key, K - 1, axis=1)[:, :K]
        ti = np.take_along_axis(
            ti, np.argsort(-np.take_along_axis(key, ti, axis=1),
                           axis=1, kind="stable"), axis=1)
        sc = np.take_along_axis(cs, ti, axis=1)
        bi = ti // W
        pi = np.take_along_axis(cpc, ti, axis=1)
        paths = np.take_along_axis(paths, bi[:, :, None], axis=1)
        paths[:, :, t] = pi.astype(np.int32)
    return paths[:, 0, :]


def kernel(cdf_map, bin_centers):
    cdf_map = np.ascontiguousarray(cdf_map, dtype=np.float32)
    bin_centers = np.asarray(bin_centers, dtype=np.float32)

    sp = run_device_sp(cdf_map)
    lnsp = np.log(sp).astype(np.float32)
    paths = _lazy_beam_search(cdf_map, lnsp)
    return bin_centers[paths]


# revision 8
# speedup vs baseline: 2.9477x; 1.4949x over previous
"""TRN2 Bass kernel for nn_DiffusionTSF (CDF beam-search decoder).

Strategy (v2 -- S'-only device pass + lazy host beam search):
 - Shard cdf_map along batch: 256 -> 8 cores x 32 (pure data parallel).
 - The beam search only ever reads K*W = 105 of the 512 bins per (b, t)
   step, at data-dependent positions -- so materializing the full
   (B, H, T) log-pdf field on device (47 MB/core written back) is
   wasted HBM traffic.  The only dense, full-input computation the
   decoder needs is the per-column normalization sum
      S'[b, t] = sum_h relu(cdf[b, h, t] - cdf[b, h+1, t]),
   a pure streaming reduction over the whole 47 MB input.  The device
   does exactly that (memory-bound: 47.2 MB read, 0.37 MB written per
   core); the host evaluates candidate log-pdf values lazily from its
   own copy of cdf_map:
      lp = max(ln(max(diff, 2^-30)) - ln S', ln EPS)
   which reproduces the reference field to f32 rounding (verified
   rel_err == 0.0 vs the jax reference on the full problem when S' is
   f32-accurate; the per-column -ln S' shift cancels in beam-search
   comparisons and only enters via the EPS clamp boundary).
 - Device layout: 128 partitions = (v: 4 h-segments) x (b: 32 batch).
   Input DMA: 4 x 32-partition dma_starts per chunk with NATURAL source
   APs (contiguous 23 KB runs) -- measured 343 GB/s/core (near the
   ~358 GB/s HBM-per-core roofline).  A single 128-partition dma_start
   with a transposed source AP measured 72 GB/s -- 4.8x slower.
 - Compute on DVE + ACT only.  Measured engine facts (HW, f32):
     DVE tensor_tensor (dual-src): ~0.9-1.6 elem/cyc/lane
     DVE tensor_scalar (single-src): ~4.6 elem/cyc/lane (4x mode)
     ACT relu: ~1.05 elem/cyc/lane; overlaps DVE when DMA is quiet
     GPSIMD: slow (tree 127 us vs 58 on DVE) AND DVE's 2-port perf
       mode locks GPSIMD out of SBUF -- never put it next to busy DVE.
     DMA does NOT overlap engine compute here (~137 us DMA + engine
       time adds almost serially); minimize TOTAL engine-busy time.
   Per chunk: DVE even/odd-split subs (disjoint operand ranges,
   ~13% faster), ACT relus the top half, DVE scalar_tensor_tensor
   fuses relu(bottom half)+level-1 add, DVE finishes the tree + acc.
 - S' stays f32 end-to-end: f32 S' reproduces the reference output
   exactly (rel_err 0.0); an f16 tree costs ~1e-3 rel err.

Learnings kept from previous sessions:
 - engine APs cannot mix base partitions on TRN2 (no partition-shifted
   operands); keep h-adjacent rows within a partition's free dims.
 - in-place engine ops on a tile are fine.
 - col field in 16-bit fails (~9e-2 rel err from tie-break flips) --
   moot now, the field is never materialized.
"""
import numpy as np
from contextlib import ExitStack

import concourse.bass as bass
import concourse.tile as tile
from concourse import bacc, mybir
from concourse.bass_utils import run_bass_kernel_spmd

f32 = mybir.dt.float32
EPS = np.float32(1e-8)
LOGEPS = np.float32(np.log(np.float32(1e-8)))
CLAMP = np.float32(2.0 ** -30)

B, B_CORE, H, T = 256, 32, 512, 720
N_CORES = 8
NSEG = 4                  # h-segments of 128 rows (partition groups)
R = 8                     # h-rows per chunk
NCHUNK = 128 // R         # chunks per segment

BEAM_WIDTH = 5
JUMP_PENALTY = np.float32(1.0)
SEARCH_RADIUS = 10

STT_FUSE = True           # stt-fused variant measured best (251 vs 283 us)

_CACHE = {}


def _build(repeat=1):
    nc = bacc.Bacc("TRN2", target_bir_lowering=False, debug=False,
                   num_devices=N_CORES)
    # Declared (b, v, c, r, t) -- a pure reshape of the (32, 512, 720)
    # C-order batch shard, H = (v: 4) x (c: 16) x (r: 8).
    cdf_d = nc.dram_tensor("cdf", [B_CORE, NSEG, NCHUNK, R, T], f32,
                           kind="ExternalInput").ap()
    acc_d = nc.dram_tensor("accs", [128, T], f32,
                           kind="ExternalOutput").ap()
    AF = mybir.ActivationFunctionType

    with tile.TileContext(nc) as tc, ExitStack() as ctx:
        pool = ctx.enter_context(tc.tile_pool(name="p", bufs=3))
        apool = ctx.enter_context(tc.tile_pool(name="a", bufs=1))
        with tc.For_i(0, repeat) as _:
            acc = apool.tile([128, T], f32, tag="acc")
            # Segment-boundary rows for the last chunk: global row
            # 128(v+1) for v<3; row 511 duplicated for v=3 (diff -> 0).
            bnd = apool.tile([128, 1, T], f32, tag="bnd")
            for v in range(3):
                nc.sync.dma_start(bnd[32 * v:32 * v + 32],
                                  cdf_d[:, v + 1, 0, 0:1, :])
            nc.sync.dma_start(bnd[96:128], cdf_d[:, 3, NCHUNK - 1, R - 1:R, :])

            def compute(c, cur, nxt0):
                # d = cdf[h] - cdf[h+1] on DVE; even/odd row split keeps
                # each instruction's operand ranges disjoint (measured
                # ~13% faster than the overlapped-range form).
                d = pool.tile([128, R, T], f32, tag="d")
                nc.vector.tensor_sub(d[:, 0:R:2, :],
                                     cur[:, 0:R:2, :], cur[:, 1:R:2, :])
                nc.vector.tensor_sub(d[:, 1:R - 1:2, :],
                                     cur[:, 1:R - 1:2, :], cur[:, 2:R:2, :])
                nc.vector.tensor_sub(d[:, R - 1:R, :],
                                     cur[:, R - 1:R, :], nxt0)
                t4 = pool.tile([128, R // 2, T], f32, tag="t4")
                if STT_FUSE:
                    # ACT relus the top half (hidden under DVE); DVE
                    # fuses relu(bottom half) + level-1 add in one stt.
                    nc.scalar.activation(d[:, 4:8, :], d[:, 4:8, :], AF.Relu)
                    nc.vector.scalar_tensor_tensor(
                        t4[:], d[:, 0:4, :], 0.0, d[:, 4:8, :],
                        mybir.AluOpType.max, mybir.AluOpType.add)
                else:
                    # ACT relus everything (overlaps with DVE); DVE adds.
                    nc.scalar.activation(d[:], d[:], AF.Relu)
                    nc.vector.tensor_add(t4[:], d[:, 0:4, :], d[:, 4:8, :])
                nc.vector.tensor_add(t4[:, 0:2, :], t4[:, 0:2, :],
                                     t4[:, 2:4, :])
                if c == 0:
                    nc.vector.tensor_add(acc[:], t4[:, 0, :], t4[:, 1, :])
                else:
                    nc.vector.tensor_add(t4[:, 0, :], t4[:, 0, :],
                                         t4[:, 1, :])
                    nc.vector.tensor_add(acc[:], acc[:], t4[:, 0, :])

            prev = None
            for c in range(NCHUNK):
                cur = pool.tile([128, R, T], f32, tag="cin")
                # 4 x 32-partition DMAs (natural APs, contiguous 23 KB
                # runs): measured 343 GB/s/core vs 72 GB/s for a single
                # 128-partition DMA with a transposed source AP.
                for v in range(NSEG):
                    nc.sync.dma_start(cur[32 * v:32 * v + 32],
                                      cdf_d[:, v, c, :, :])
                if prev is not None:
                    compute(c - 1, prev, cur[:, 0:1, :])
                prev = cur
            compute(NCHUNK - 1, prev, bnd[:])
            nc.sync.dma_start(acc_d[:], acc[:])
    nc.compile()
    return nc


def _get_kernel(repeat=1):
    if repeat not in _CACHE:
        _CACHE[repeat] = _build(repeat)
    return _CACHE[repeat]


def run_device_sp(cdf_map, repeat=1):
    """cdf_map (256, 512, 720) f32 -> S' (256, 720) f32 (clipped at EPS)."""
    nc = _get_kernel(repeat)
    cdf_map = np.ascontiguousarray(cdf_map, dtype=np.float32)
    shards = np.split(cdf_map, N_CORES, axis=0)
    in_maps = [{"cdf": s.reshape(B_CORE, NSEG, NCHUNK, R, T)} for s in shards]
    res = run_bass_kernel_spmd(nc, in_maps, list(range(N_CORES)))
    # acc partitions: p = 32 v + b -> reshape (4, 32, T), sum over v
    sp = np.stack([res.results[i]["accs"].reshape(NSEG, B_CORE, T)
                   .sum(axis=0, dtype=np.float32) for i in range(N_CORES)])
    sp = sp.reshape(N_CORES * B_CORE, T)
    return np.clip(sp, EPS, None)


def _lazy_beam_search(cdf, lnsp):
    """Beam search over the implicit lp field; candidate values are
    computed on the fly from cdf (B, H, T) and lnsp (B, T) = ln(S').
    Exact replica of the reference dynamics incl. stable top-k
    tie-breaking (ties -> ascending flat index).  Returns (B, T) int32
    paths of the rank-0 beam."""
    K = BEAM_WIDTH
    W = 2 * SEARCH_RADIUS + 1
    offs = np.arange(-SEARCH_RADIUS, SEARCH_RADIUS + 1)
    pen = (JUMP_PENALTY * np.abs(offs)).astype(np.float32)
    bidx = np.arange(B)[:, None]

    cdfT = np.ascontiguousarray(cdf.transpose(2, 0, 1))  # (T, B, H)

    # t = 0: full-column lp for the init top-k
    d0 = cdf[:, :-1, 0] - cdf[:, 1:, 0]
    col0 = np.log(np.maximum(d0, CLAMP).astype(np.float32))
    col0 = np.concatenate([col0, np.full((B, 1), -100.0, np.float32)], axis=1)
    lp0 = np.maximum(col0 - lnsp[:, 0:1], LOGEPS)
    ord0 = np.argsort(-lp0, axis=1, kind="stable")[:, :K]
    sc = np.take_along_axis(lp0, ord0, axis=1)
    paths = np.zeros((B, K, T), dtype=np.int32)
    paths[:, :, 0] = ord0
    kidx = np.arange(K * W, dtype=np.float64)
    for t in range(1, T):
        prev = paths[:, :, t - 1]
        cand = prev[:, :, None] + offs[None, None, :]
        valid = (cand >= 0) & (cand < H)
        cpc = np.clip(cand, 0, H - 1).reshape(B, -1)          # (B, K*W)
        slab = cdfT[t]                                        # (B, H)
        c0 = slab[bidx, cpc]
        c1 = slab[bidx, np.minimum(cpc + 1, H - 1)]
        diff = np.where(cpc == H - 1, np.float32(0.0), c0 - c1)
        colv = np.log(np.maximum(diff, CLAMP).astype(np.float32))
        lpv = np.maximum(colv - lnsp[:, t, None], LOGEPS).reshape(B, K, W)
        cs = (sc[:, :, None] + lpv) - pen[None, None, :]
        cs = np.where(valid, cs, -np.float32(np.inf)).reshape(B, -1)
        # f64 key: score * 2^30 - flat_index; |score| >= 4 so distinct
        # f32 scores stay distinct and ties break toward the lowest
        # index, exactly like lax.top_k on the raw scores.
        key = np.where(np.isneginf(cs), -1e30, cs.astype(np.float64))
        key = key * np.float64(2.0 ** 30) - kidx[None, :]
        ti = np.argpartition(-key, K - 1, axis=1)[:, :K]
        ti = np.take_along_axis(
            ti, np.argsort(-np.take_along_axis(key, ti, axis=1),
                           axis=1, kind="stable"), axis=1)
        sc = np.take_along_axis(cs, ti, axis=1)
        bi = ti // W
        pi = np.take_along_axis(cpc, ti, axis=1)
        paths = np.take_along_axis(paths, bi[:, :, None], axis=1)
        paths[:, :, t] = pi.astype(np.int32)
    return paths[:, 0, :]


def kernel(cdf_map, bin_centers):
    cdf_map = np.ascontiguousarray(cdf_map, dtype=np.float32)
    bin_centers = np.asarray(bin_centers, dtype=np.float32)

    sp = run_device_sp(cdf_map)
    lnsp = np.log(sp).astype(np.float32)
    paths = _lazy_beam_search(cdf_map, lnsp)
    return bin_centers[paths]
